# revision 1
# baseline (speedup 1.0000x reference)
"""Trainium2 Bass kernel for nn_AttentionLayer (B=4, S=2048, H=16, DH=64).

Sharding: 8 cores = 4 batches x 2 head-halves. Core c handles batch c//2,
heads (c%2)*8 .. (c%2)*8+8 (i.e. 512 of the 1024 QKV columns).

Per-core device program (SPMD, same program on all cores, different inputs):
  inputs (pre-laid-out on host):
    xT  [1024, 2048]  = x[b].T           (contraction dim on partitions)
    wq/wk/wv [1024, 512]                 (column slice for this core's heads)
    bq/bk/bv [512]
  output:
    out [512, 2048] = attention ctx for this core's 8 heads, transposed
                      (head*64+dh on rows, seq on cols); host transposes back.

Structure (single TileContext; all matmuls fp32r = full PE rate, ~1e-4 rel
error; Tile schedules by dependency + priority):
  - Priority bands: every attention-unit instruction outranks the QKV/V
    "filler" work, so the exp pipeline never starves while projections
    gap-fill the PE between attention matmuls.
  - V pass: V = x@Wv (PE) + bv (DVE add) -> v_aug [128, 8, 65] x16 seq tiles
    (col 64 = ones; the 65th PV output row accumulates the softmax
    denominator for free).
  - Per head-pair m: QT/KT = (x@W)^T on PE + bias via tensor_scalar_add on
    the PSUM->SBUF copy (weights streamed per pair, x re-streamed from HBM;
    Q chunks 2,3 deferred past qg0 to rebalance PE). Then attention units
    (m, qgroup of 1024 q, head):
      per k-block kb: scoresT [128,1024] = KT_chunk.T @ QT (PE, K=64);
                      E = exp(scoresT/8) (ACT, PSUM->SBUF, f32r out);
                      ctxT [65,512]x2 += v_aug.T @ E (PE, PSUM-accumulated)
      then per 512-q half: copy ctx to SBUF (frees PSUM), reciprocal of row
      64 (DVE), partition-broadcast (GPSIMD), multiply (DVE), DMA out.
  Emission is software-pipelined two scores-blocks ahead of PV so the next
  exp input always wins the PE race against the previous PV; the first
  attention unit is split 512-wide to start exp as soon as QK chunk 0 lands.
  The final unit normalizes directly from PSUM (no successor needs its ctx
  slot, so the early-release copy is skipped there).
  PSUM: scores 2x2 banks + ctx 2x1 + qkv 2 = 8. Modeled (TimelineSim)
  per-core time ~332 us; ACT (exp) busy ~267 us, PE busy ~305 us.
"""

import numpy as np

B, S, H, DH = 4, 2048, 16, 64
D = H * DH  # 1024
NCORES = 8
COLS = 512  # qkv columns per core (8 heads)
NPAIR = 4  # head pairs per core
NKB = S // 128  # 16 k-blocks
QG = 1024  # q-group width
NQG = S // QG  # 2
XC = 512  # x streaming chunk (seq cols)
NXC = S // XC  # 4
INV_SQRT_DH = 1.0 / 8.0

_CACHE = {}


def _build():
    import concourse.mybir as mybir
    import concourse.tile as tile
    from concourse import bacc

    f32 = mybir.dt.float32
    f32r = mybir.dt.float32r
    Exp = mybir.ActivationFunctionType.Exp

    nc = bacc.Bacc(
        "TRN2",
        target_bir_lowering=False,
        debug=False,
        enable_asserts=False,
        num_devices=NCORES,
    )

    xT_d = nc.dram_tensor("xT", [D, S], f32r, kind="ExternalInput").ap()
    wq_d = nc.dram_tensor("wq", [D, COLS], f32r, kind="ExternalInput").ap()
    wk_d = nc.dram_tensor("wk", [D, COLS], f32r, kind="ExternalInput").ap()
    wv_d = nc.dram_tensor("wv", [D, COLS], f32r, kind="ExternalInput").ap()
    bq_d = nc.dram_tensor("bq", [COLS], f32, kind="ExternalInput").ap()
    bk_d = nc.dram_tensor("bk", [COLS], f32, kind="ExternalInput").ap()
    bv_d = nc.dram_tensor("bv", [COLS], f32, kind="ExternalInput").ap()
    out_d = nc.dram_tensor("out", [COLS, S], f32, kind="ExternalOutput").ap()

    with tile.TileContext(nc) as tc:
        with (
            tc.tile_pool(name="consts", bufs=1) as consts,
            tc.tile_pool(name="vpool", bufs=1) as vpool,
            tc.tile_pool(name="wvpool", bufs=1) as wvpool,
            tc.tile_pool(name="wqk", bufs=2) as wqk,
            tc.tile_pool(name="xpool", bufs=2) as xpool,
            tc.tile_pool(name="qkt", bufs=2) as qkt,
            tc.tile_pool(name="epool", bufs=12) as epool,
            tc.tile_pool(name="opool", bufs=2) as opool,
            tc.tile_pool(name="psum", bufs=1, space="PSUM") as psum,
        ):
            # ---- constants, ACT table preload ----
            bq_t = consts.tile([128, NPAIR], f32)
            bk_t = consts.tile([128, NPAIR], f32)
            bv_s = consts.tile([1, COLS], f32)
            bvb = consts.tile([128, COLS], f32)
            nc.gpsimd.dma_start(out=bq_t, in_=bq_d.rearrange("(m p) -> p m", p=128))
            nc.gpsimd.dma_start(out=bk_t, in_=bk_d.rearrange("(m p) -> p m", p=128))
            nc.gpsimd.dma_start(out=bv_s, in_=bv_d[None, :])
            nc.gpsimd.partition_broadcast(bvb, bv_s)
            warm = consts.tile([1, 1], f32)
            nc.vector.memset(warm, 0.0)
            nc.scalar.activation(warm, warm, Exp)  # pull ACT table load early

            vt = [vpool.tile([128, 8, 65], f32r, name=f"vt{i}") for i in range(NKB)]
            for i in range(NKB):
                nc.vector.memset(vt[i][:, :, 64:65].bitcast(f32), 1.0)

            wv = wvpool.tile([128, 8, COLS], f32r, name="wv")

            def load_wv():
                nc.sync.dma_start(
                    out=wv, in_=wv_d.rearrange("(j p) c -> p j c", p=128)
                )

            def load_x_chunk(c, wpair=None, eng=None):
                if wpair is not None:
                    load_w_dma(*wpair)
                if eng is None:
                    eng = nc.sync
                xt = xpool.tile([128, 8, XC], f32r, name="xt", tag="xt")
                for j in range(8):
                    eng.dma_start(
                        out=xt[:, j, :],
                        in_=xT_d[j * 128 : (j + 1) * 128, c * XC : (c + 1) * XC],
                    )
                return xt

            def v_pass(chunks):
                for c in chunks:
                    xt = load_x_chunk(c)
                    for i in range(XC // 128):
                        it = c * (XC // 128) + i
                        ps = psum.tile([128, 512], f32, tag="p1", bufs=2)
                        for j in range(8):
                            nc.tensor.matmul(
                                ps,
                                lhsT=xt[:, j, i * 128 : (i + 1) * 128],
                                rhs=wv[:, j, :],
                                start=(j == 0),
                                stop=(j == 7),
                            )
                        nc.vector.tensor_add(
                            vt[it][:, :, 0:64],
                            ps.rearrange("p (h d) -> p h d", h=8),
                            bvb.rearrange("p (h d) -> p h d", h=8),
                        )

            def load_w_dma(wqm, wkm, m):
                nc.sync.dma_start(
                    out=wkm,
                    in_=wk_d[:, m * 128 : (m + 1) * 128].rearrange(
                        "(j p) c -> p j c", p=128
                    ),
                )
                nc.sync.dma_start(
                    out=wqm,
                    in_=wq_d[:, m * 128 : (m + 1) * 128].rearrange(
                        "(j p) c -> p j c", p=128
                    ),
                )

            def load_w_pair(m, defer_dma=False):
                wqm = wqk.tile([128, 8, 128], f32r, name="wqm", tag="wqm")
                wkm = wqk.tile([128, 8, 128], f32r, name="wkm", tag="wkm")
                if not defer_dma:
                    load_w_dma(wqm, wkm, m)
                return wqm, wkm

            def qk_chunk(m, c, xt, wqm, wkm, qt, kt, projs=("k", "q")):
                pairs = {"q": (wqm, bq_t, qt), "k": (wkm, bk_t, kt)}
                for w, bias, dst in (pairs[p] for p in projs):
                    ps = psum.tile([128, 512], f32, tag="p1", bufs=2)
                    for j in range(8):
                        nc.tensor.matmul(
                            ps,
                            lhsT=w[:, j, :],
                            rhs=xt[:, j, :],
                            start=(j == 0),
                            stop=(j == 7),
                        )
                    nc.vector.tensor_scalar_add(
                        dst[:, c * XC : (c + 1) * XC], ps, bias[:, m : m + 1]
                    )

            def v_chunk(c, xt):
                for i in range(XC // 128):
                    it = c * (XC // 128) + i
                    ps = psum.tile([128, 512], f32, tag="p1", bufs=2)
                    for j in range(8):
                        nc.tensor.matmul(
                            ps,
                            lhsT=xt[:, j, i * 128 : (i + 1) * 128],
                            rhs=wv[:, j, :],
                            start=(j == 0),
                            stop=(j == 7),
                        )
                    nc.vector.tensor_add(
                        vt[it][:, :, 0:64],
                        ps.rearrange("p (h d) -> p h d", h=8),
                        bvb.rearrange("p (h d) -> p h d", h=8),
                    )

            def emit_attention_unit(m, qg, h, q0, p0, head, qt, kt, qw=QG, last=False):
                nq = qw // 512
                ctx = [
                    psum.tile([65, 512], f32, tag="ctx", bufs=2, name=f"ctx{qq}")
                    for qq in range(nq)
                ]

                def scores(kb):
                    sc = psum.tile([128, qw], f32, tag="sc", bufs=2)
                    for qq in range(nq):
                        nc.tensor.matmul(
                            sc[:, qq * 512 : (qq + 1) * 512],
                            lhsT=kt[p0 : p0 + 64, kb * 128 : (kb + 1) * 128],
                            rhs=qt[
                                p0 : p0 + 64,
                                q0 + qq * 512 : q0 + (qq + 1) * 512,
                            ],
                            start=True,
                            stop=True,
                        )
                    return sc

                # software-pipelined emission, two scores ahead: at exp(kb)'s
                # end both PV(kb) and scores(kb+2) become ready on PE; the
                # scores must win that race (higher priority = emitted
                # earlier) or exp(kb+2) slips past exp(kb+1)'s window.
                scs = [scores(0), scores(1)]
                for kb in range(NKB):
                    ee = epool.tile([128, qw], f32r, tag="e")
                    nc.scalar.activation(ee, scs[kb % 2], Exp, scale=INV_SQRT_DH)
                    if kb < NKB - 2:
                        scs[kb % 2] = scores(kb + 2)
                    for qq in range(nq):
                        nc.tensor.matmul(
                            ctx[qq],
                            lhsT=vt[kb][:, head, :],
                            rhs=ee[:, qq * 512 : (qq + 1) * 512],
                            start=(kb == 0),
                            stop=(kb == NKB - 1),
                        )
                for qq in range(nq):
                    if last:
                        src_t = ctx[qq]
                    else:
                        cs = opool.tile([65, 512], f32, tag="cs")
                        nc.vector.tensor_copy(cs, ctx[qq])
                        src_t = cs
                    rr = opool.tile([1, 512], f32, tag="r")
                    nc.vector.reciprocal(rr, src_t[64:65, :])
                    rb = opool.tile([64, 512], f32, tag="rb")
                    nc.gpsimd.partition_broadcast(rb, rr)
                    ob = opool.tile([64, 512], f32, tag="o")
                    nc.vector.tensor_mul(ob, src_t[0:64, :], rb)
                    nc.sync.dma_start(
                        out=out_d[
                            head * 64 : (head + 1) * 64,
                            q0 + qq * 512 : q0 + (qq + 1) * 512,
                        ],
                        in_=ob,
                    )

            # ---- banded priorities: attention preferred, QKV/V fill gaps ----
            from contextlib import contextmanager

            base = tc.cur_priority + 50
            att_cur = [base]
            fill_cur = [base + 6000]

            @contextmanager
            def band(cursor):
                off = tc.cur_priority - cursor[0]
                with tc.high_priority(offset=off):
                    yield
                    cursor[0] = tc.cur_priority

            # ---- per pair: QKV (filler band) then attention (att band) ----
            for m in range(NPAIR):
                with band(fill_cur):
                    wqm, wkm = load_w_pair(m, defer_dma=(m == 0))
                    qt = qkt.tile([128, S], f32r, name=f"qt{m}", tag="qt")
                    kt = qkt.tile([128, S], f32r, name=f"kt{m}", tag="kt")
                    xts = {}
                    if m == 0:
                        xts[0] = load_x_chunk(0, wpair=(wqm, wkm, m))
                        xts[1] = load_x_chunk(1)
                        qk_chunk(m, 0, xts[0], wqm, wkm, qt, kt)
                        qk_chunk(m, 1, xts[1], wqm, wkm, qt, kt)
                        load_wv()
                        v_chunk(0, xts[0])
                        v_chunk(1, xts[1])
                        for c in (2, 3):
                            xts[c] = load_x_chunk(c)
                            qk_chunk(m, c, xts[c], wqm, wkm, qt, kt)
                            v_chunk(c, xts[c])
                    else:
                        for c in range(NXC):
                            xts[c] = load_x_chunk(c)
                            qk_chunk(
                                m, c, xts[c], wqm, wkm, qt, kt,
                                projs=("q", "k") if c < 2 else ("k",),
                            )

                # ---- attention units (Q c2/c3 deferred after qg0) ----
                for qg in range(NQG):
                    q0 = qg * QG
                    for h in range(2):
                        head = 2 * m + h
                        p0 = h * 64
                        with band(att_cur):
                            if m == 0 and qg == 0 and h == 0:
                                emit_attention_unit(
                                    m, qg, h, q0, p0, head, qt, kt, qw=512
                                )
                                emit_attention_unit(
                                    m, qg, h, q0 + 512, p0, head, qt, kt, qw=512
                                )
                            else:
                                emit_attention_unit(
                                    m, qg, h, q0, p0, head, qt, kt,
                                    last=(m == NPAIR - 1 and qg == NQG - 1 and h == 1),
                                )
                    if qg == 0 and m > 0:
                        with band(fill_cur):
                            for c in (2, 3):
                                qk_chunk(m, c, xts[c], wqm, wkm, qt, kt, projs=("q",))


    nc.compile()
    return nc


def _get_nc():
    if "nc" not in _CACHE:
        _CACHE["nc"] = _build()
    return _CACHE["nc"]


def _in_maps(x, Wq, bq, Wk, bk, Wv, bv):
    maps = []
    for c in range(NCORES):
        b, hh = c // 2, c % 2
        cs = slice(hh * COLS, (hh + 1) * COLS)
        maps.append(
            {
                "xT": np.ascontiguousarray(np.asarray(x)[b].T),
                "wq": np.ascontiguousarray(np.asarray(Wq)[:, cs]),
                "wk": np.ascontiguousarray(np.asarray(Wk)[:, cs]),
                "wv": np.ascontiguousarray(np.asarray(Wv)[:, cs]),
                "bq": np.ascontiguousarray(np.asarray(bq)[cs]),
                "bk": np.ascontiguousarray(np.asarray(bk)[cs]),
                "bv": np.ascontiguousarray(np.asarray(bv)[cs]),
            }
        )
    return maps


def _run(inputs, trace=False):
    from concourse import bass_utils

    nc = _get_nc()
    res = bass_utils.run_bass_kernel_spmd(
        nc,
        _in_maps(**inputs),
        core_ids=list(range(NCORES)),
        trace=trace,
    )
    out = np.empty((B, S, D), np.float32)
    for c in range(NCORES):
        b, hh = c // 2, c % 2
        out[b, :, hh * COLS : (hh + 1) * COLS] = res.results[c]["out"].T
    return out, res


def kernel(**inputs):
    out, _ = _run(inputs, trace=False)
    return out


if __name__ == "__main__":
    _get_nc()
    print("build ok")



# revision 7
# speedup vs baseline: 1.0372x; 1.0372x over previous
"""Trainium2 Bass kernel for nn_AttentionLayer (B=4, S=2048, H=16, DH=64).

Sharding: 8 cores = 4 batches x 2 head-halves. Core c handles batch c//2,
heads (c%2)*8 .. (c%2)*8+8 (512 of the 1024 QKV columns).

Per-core structure (ACT-bound design, ~256us modeled):
  - Q/K projections (PE, fp32r per head-pair column chunk) write q/k
    transposed as bf16 via the DVE bias-add: qt/kt[m] [128p = 2 heads x
    64 dh, 2048 s].
  - V projection (PE, fp32r) -> vt[kb] [128 kpos, 8 heads, 65] bf16 with
    col 64 = 1.0 (PV accumulates the softmax denominator for free).
  - Attention stream: 512 slots; group = (head, 512-q block) with qb-minor
    group order (all heads' qb0 first) so q s-chunk demand is spread; slots
    = 16 k-blocks per group. Slots are packed into alternating 3-slot/2-slot
    PSUM score tiles; one exp (ACT) per tile (1536/1024 wide) is the
    metronome: ACT busy ~= 256us, PE ~= 247us.
  - Scores: bf16 matmul [64 dh contraction] -> scoresT [128 kpos, 512 q]
    per slot (bf16 q,k + bf16 E/V measured ~0.3% rel RMS; fp8 DoubleRow
    measured 2% = over the gate, hence bf16).
  - PV reoriented to ctx[q, d]: out [128 q, 65] per (slot, qtile) in bf16,
    accumulated over kb into a memset-zeroed PSUM bank per group
    (start=False + skip_group_check so 4 sub-bank groups share one bank).
  - Normalization: DVE reciprocal of ctx col 64 + per-qtile scalar multiply,
    DMA out via the Pool sequencer.
  PSUM: qkv 2 banks + scores 3+2 banks + ctx 1 bank = 8.
  Tile derives data dependencies from emission order, so all projections are
  emitted (low-priority fill band) before the attention stream; priorities
  interleave them into PE gaps at schedule time.
"""

import numpy as np

B, S, H, DH = 4, 2048, 16, 64
D = H * DH  # 1024
NCORES = 8
COLS = 512  # qkv columns per core (8 heads)
NKB = 16
EXP_SCALE = 0.125  # 1/sqrt(DH)

_CACHE = {}


def _build():
    import concourse.mybir as mybir
    import concourse.tile as tile
    from concourse import bacc

    f32 = mybir.dt.float32
    f32r = mybir.dt.float32r
    bf16 = mybir.dt.bfloat16
    Exp = mybir.ActivationFunctionType.Exp

    nc = bacc.Bacc(
        "TRN2",
        target_bir_lowering=False,
        debug=False,
        enable_asserts=False,
        num_devices=NCORES,
    )

    xT_d = nc.dram_tensor("xT", [D, S], f32r, kind="ExternalInput").ap()
    wq_d = nc.dram_tensor("wq", [D, COLS], f32r, kind="ExternalInput").ap()
    wk_d = nc.dram_tensor("wk", [D, COLS], f32r, kind="ExternalInput").ap()
    wv_d = nc.dram_tensor("wv", [D, COLS], f32r, kind="ExternalInput").ap()
    bq_d = nc.dram_tensor("bq", [COLS], f32, kind="ExternalInput").ap()
    bk_d = nc.dram_tensor("bk", [COLS], f32, kind="ExternalInput").ap()
    bv_d = nc.dram_tensor("bv", [COLS], f32, kind="ExternalInput").ap()
    out_d = nc.dram_tensor("out", [S, COLS], f32, kind="ExternalOutput").ap()

    with tile.TileContext(nc) as tc:
        with (
            tc.tile_pool(name="consts", bufs=1) as consts,
            tc.tile_pool(name="wpool", bufs=1) as wpool,
            tc.tile_pool(name="qkp", bufs=1) as qkp,
            tc.tile_pool(name="vpool", bufs=1) as vpool,
            tc.tile_pool(name="xpool", bufs=1) as xpool,
            tc.tile_pool(name="epool", bufs=1) as epool,
            tc.tile_pool(name="opool", bufs=1) as opool,
            tc.tile_pool(name="psum", bufs=1, space="PSUM") as psum,
        ):
            from contextlib import contextmanager

            base = tc.cur_priority + 50
            att_cur = [base]
            fill_cur = [base + 8000]

            @contextmanager
            def band(cursor):
                off = tc.cur_priority - cursor[0]
                with tc.high_priority(offset=off):
                    yield
                    cursor[0] = tc.cur_priority

            # ---- constants ----
            with band(att_cur):
                warm = consts.tile([1, 1], f32)
                nc.vector.memset(warm, 0.0)
                nc.scalar.activation(warm, warm, Exp)  # pull ACT table load early

            with band(fill_cur):
                bq_t = consts.tile([128, 4], f32)
                bk_t = consts.tile([128, 4], f32)
                bv_s = consts.tile([1, COLS], f32)
                bvb = consts.tile([128, COLS], f32)
                nc.gpsimd.dma_start(out=bq_t, in_=bq_d.rearrange("(m p) -> p m", p=128))
                nc.gpsimd.dma_start(out=bk_t, in_=bk_d.rearrange("(m p) -> p m", p=128))
                nc.gpsimd.dma_start(out=bv_s, in_=bv_d[None, :])
                nc.gpsimd.partition_broadcast(bvb, bv_s)

                vt = [vpool.tile([128, 8, 65], bf16, name=f"vt{i}") for i in range(NKB)]
                for i in range(NKB):
                    nc.vector.memset(vt[i][:, :, 64:65], 1.0)

                wv_t = wpool.tile([128, 8, COLS], f32r, name="wv_t")

                # x chunks: all four resident
                xt = []
                for c in range(4):
                    xc = xpool.tile([128, 8, 512], f32r, name=f"xt{c}")
                    eng = nc.sync if c == 0 else nc.gpsimd
                    for j in range(8):
                        eng.dma_start(
                            out=xc[:, j, :],
                            in_=xT_d[j * 128 : (j + 1) * 128, c * 512 : (c + 1) * 512],
                        )
                    xt.append(xc)

                nc.gpsimd.dma_start(
                    out=wv_t, in_=wv_d.rearrange("(j p) c -> p j c", p=128)
                )

                # bf16 q/k transposed tiles per head pair m:
                # [128p = 2 heads x 64 dh, 2048 s]
                qt_b = [qkp.tile([128, S], bf16, name=f"qt{m}") for m in range(4)]
                kt_b = [qkp.tile([128, S], bf16, name=f"kt{m}") for m in range(4)]

            # ---- weight chunk ring (2 bufs per proj; quad1 reuses quad0's) ----
            wcur = {"q": {}, "k": {}}

            def load_w(proj, m, eng):
                w_d = wq_d if proj == "q" else wk_d
                wt = wpool.tile(
                    [128, 8, 128], f32r, tag=f"w{proj}", bufs=2, name=f"w{proj}{m}"
                )
                eng.dma_start(
                    out=wt,
                    in_=w_d[:, m * 128 : (m + 1) * 128].rearrange(
                        "(j p) c -> p j c", p=128
                    ),
                )
                wcur[proj][m] = wt

            with band(fill_cur):
                load_w("k", 0, nc.sync)
                load_w("k", 1, nc.sync)
                load_w("q", 0, nc.sync)
                load_w("q", 1, nc.sync)

            # ---- projection unit emitters (fill band) ----
            def proj_qk(proj, m, c):
                dst = qt_b[m] if proj == "q" else kt_b[m]
                bias_t = bq_t if proj == "q" else bk_t
                w = wcur[proj][m]
                ps = psum.tile([128, 512], f32, tag="qkv", bufs=2, name="psq")
                for j in range(8):
                    nc.tensor.matmul(
                        ps,
                        lhsT=w[:, j, :],
                        rhs=xt[c][:, j, :],
                        start=(j == 0),
                        stop=(j == 7),
                    )
                nc.vector.tensor_scalar_add(
                    dst[:, c * 512 : (c + 1) * 512], ps, bias_t[:, m : m + 1]
                )

            def proj_v(Q, c, i):
                # V for head-quad Q, s-chunk c, seq subchunk i -> vt[4c+i]
                ps = psum.tile([128, 512], f32, tag="qkv", bufs=2, name="psv")
                for j in range(8):
                    nc.tensor.matmul(
                        ps[:, 0:256],
                        lhsT=xt[c][:, j, i * 128 : (i + 1) * 128],
                        rhs=wv_t[:, j, Q * 256 : (Q + 1) * 256],
                        start=(j == 0),
                        stop=(j == 7),
                    )
                nc.vector.tensor_add(
                    vt[4 * c + i][:, 4 * Q : 4 * Q + 4, 0:64],
                    ps[:, 0:256].rearrange("p (h d) -> p h d", h=4),
                    bvb[:, Q * 256 : (Q + 1) * 256].rearrange("p (h d) -> p h d", h=4),
                )

            # projection emission order (consumption-ordered)
            proj_order = []
            proj_order += [("k", 0, 0), ("q", 0, 0), ("k", 1, 0), ("q", 1, 0)]
            proj_order += [("k", m, c) for c in (1, 2, 3) for m in (0, 1)]
            proj_order += [("q", m, 1) for m in (0, 1)]
            proj_order += [("v", 0, c, i) for c in range(4) for i in range(4)]
            proj_order += [("q", m, c) for c in (2, 3) for m in (0, 1)]
            proj_order += [("wl", "k", 2), ("wl", "k", 3), ("wl", "q", 2), ("wl", "q", 3)]
            proj_order += [("k", m, c) for c in range(4) for m in (2, 3)]
            proj_order += [("q", m, 0) for m in (2, 3)]
            proj_order += [("v", 1, c, i) for c in range(4) for i in range(4)]
            proj_order += [("q", m, c) for c in (1, 2, 3) for m in (2, 3)]

            def emit_proj_all():
                with band(fill_cur):
                    for u in proj_order:
                        if u[0] == "wl":
                            load_w(u[1], u[2], nc.gpsimd)
                        elif u[0] == "v":
                            proj_v(u[1], u[2], u[3])
                        else:
                            proj_qk(u[0], u[1], u[2])

            # ---- attention stream ----
            # group order: quad-major, qb-minor: all 4 heads of a quad at qb,
            # then next qb
            groups = [
                (4 * Q + h4, qb)
                for Q in range(2)
                for qb in range(4)
                for h4 in range(4)
            ]
            slots = [(h, qb, kb) for (h, qb) in groups for kb in range(NKB)]
            units = []
            pos = 0
            widths = [3, 2]
            ui = 0
            while pos < len(slots):
                w = min(widths[ui % 2], len(slots) - pos)
                units.append(slots[pos : pos + w])
                pos += w
                ui += 1

            ctx_cur = [None]

            def emit_scores(u):
                unit = units[u]
                full = widths[u % 2]
                tag = "scA" if u % 2 == 0 else "scB"
                with band(att_cur):
                    sc = psum.tile([128, full, 512], f32, tag=tag, bufs=1, name="sc")
                    for i, (h, qb, kb) in enumerate(unit):
                        m, p0 = h // 2, 64 * (h % 2)
                        nc.tensor.matmul(
                            sc[:, i, :],
                            lhsT=kt_b[m][p0 : p0 + 64, kb * 128 : (kb + 1) * 128],
                            rhs=qt_b[m][p0 : p0 + 64, qb * 512 : (qb + 1) * 512],
                            start=True,
                            stop=True,
                        )
                return sc

            def emit_exp(u, sc):
                w = len(units[u])
                with band(att_cur):
                    ee = epool.tile([128, 3, 512], bf16, tag="e", bufs=12, name="ee")
                    nc.scalar.activation(
                        ee[:, 0:w, :], sc[:, 0:w, :], Exp, scale=EXP_SCALE
                    )
                return ee

            def emit_pv(u, ee):
                unit = units[u]
                with band(att_cur):
                    for i, (h, qb, kb) in enumerate(unit):
                        if kb == 0:
                            ctx_cur[0] = psum.tile(
                                [128, 4, 65], f32, tag="ctx", bufs=1, name="ctx"
                            )
                            nc.vector.memset(ctx_cur[0], 0.0)
                        ctx = ctx_cur[0]
                        for qt in range(4):
                            nc.tensor.matmul(
                                ctx[:, qt, :],
                                lhsT=ee[:, i, qt * 128 : (qt + 1) * 128],
                                rhs=vt[kb][:, h, :],
                                start=False,
                                stop=False,
                                skip_group_check=True,
                            )
                        if kb == NKB - 1:
                            rr = opool.tile([128, 4, 1], f32, tag="r", bufs=3, name="rr")
                            nc.vector.reciprocal(rr, ctx[:, :, 64:65])
                            ob = opool.tile([128, 4, 64], f32, tag="o", bufs=3, name="ob")
                            for qt in range(4):
                                nc.vector.tensor_scalar_mul(
                                    ob[:, qt, :],
                                    ctx[:, qt, 0:64],
                                    rr[:, qt : qt + 1, 0:1],
                                )
                            nc.gpsimd.dma_start(
                                out=out_d[
                                    qb * 512 : (qb + 1) * 512, h * 64 : (h + 1) * 64
                                ].rearrange("(t p) d -> p t d", p=128),
                                in_=ob,
                            )

            emit_proj_all()
            NU = len(units)
            scs = {0: emit_scores(0), 1: emit_scores(1)}
            for u in range(NU):
                ee = emit_exp(u, scs.pop(u))
                if u + 2 < NU:
                    scs[u + 2] = emit_scores(u + 2)
                emit_pv(u, ee)

    nc.compile()
    return nc


def _get_nc():
    if "nc" not in _CACHE:
        _CACHE["nc"] = _build()
    return _CACHE["nc"]


def _in_maps(x, Wq, bq, Wk, bk, Wv, bv):
    x = np.asarray(x, np.float32)
    maps = []
    for c in range(NCORES):
        b, hh = c // 2, c % 2
        cs = slice(hh * COLS, (hh + 1) * COLS)
        maps.append(
            {
                "xT": np.ascontiguousarray(x[b].T),
                "wq": np.ascontiguousarray(np.asarray(Wq, np.float32)[:, cs]),
                "wk": np.ascontiguousarray(np.asarray(Wk, np.float32)[:, cs]),
                "wv": np.ascontiguousarray(np.asarray(Wv, np.float32)[:, cs]),
                "bq": np.ascontiguousarray(np.asarray(bq, np.float32)[cs]),
                "bk": np.ascontiguousarray(np.asarray(bk, np.float32)[cs]),
                "bv": np.ascontiguousarray(np.asarray(bv, np.float32)[cs]),
            }
        )
    return maps


def _run(inputs, trace=False):
    from concourse import bass_utils

    nc = _get_nc()
    res = bass_utils.run_bass_kernel_spmd(
        nc,
        _in_maps(**inputs),
        core_ids=list(range(NCORES)),
        trace=trace,
    )
    out = np.empty((B, S, D), np.float32)
    for c in range(NCORES):
        b, hh = c // 2, c % 2
        out[b, :, hh * COLS : (hh + 1) * COLS] = res.results[c]["out"]
    return out, res


def kernel(**inputs):
    out, _ = _run(inputs, trace=False)
    return out


if __name__ == "__main__":
    _get_nc()
    print("build ok")


# revision 20
# speedup vs baseline: 1.1371x; 1.0963x over previous
"""Trainium2 Bass kernel for nn_AttentionLayer (B=4, S=2048, H=16, DH=64).

Sharding: 8 cores = 4 batches x 2 head-halves. Core c handles batch c//2,
heads (c%2)*8 .. (c%2)*8+8 (512 of the 1024 QKV columns).

Per-core structure (ACT-bound design, ~256us modeled):
  - Q/K projections (PE, fp32r per head-pair column chunk) write q/k
    transposed as bf16 via the DVE bias-add: qt/kt[m] [128p = 2 heads x
    64 dh, 2048 s].
  - V projection (PE, fp32r) -> vt[kb] [128 kpos, 8 heads, 65] bf16 with
    col 64 = 1.0 (PV accumulates the softmax denominator for free).
  - Attention stream: 512 slots; group = (head, 512-q block) with qb-minor
    group order (all heads' qb0 first) so q s-chunk demand is spread; slots
    = 16 k-blocks per group. Slots are packed into alternating 3-slot/2-slot
    PSUM score tiles; one exp (ACT) per tile (1536/1024 wide) is the
    metronome: ACT busy ~= 256us, PE ~= 247us.
  - Scores: bf16 matmul [64 dh contraction] -> scoresT [128 kpos, 512 q]
    per slot (bf16 q,k + bf16 E/V measured ~0.3% rel RMS; fp8 DoubleRow
    measured 2% = over the gate, hence bf16).
  - PV reoriented to ctx[q, d]: out [128 q, 65] per (slot, qtile) in bf16,
    accumulated over kb into a memset-zeroed PSUM bank per group
    (start=False + skip_group_check so 4 sub-bank groups share one bank).
  - Normalization: DVE reciprocal of ctx col 64 + per-qtile scalar multiply,
    DMA out via the Pool sequencer.
  PSUM: qkv 2 banks + scores 3+2 banks + ctx 1 bank = 8.
  Tile derives data dependencies from emission order, so all projections are
  emitted (low-priority fill band) before the attention stream; priorities
  interleave them into PE gaps at schedule time.
"""

import numpy as np

B, S, H, DH = 4, 2048, 16, 64
D = H * DH  # 1024
NCORES = 8
COLS = 512  # qkv columns per core (8 heads)
NKB = 16
EXP_SCALE = 0.125  # 1/sqrt(DH)

_CACHE = {}


def _build():
    import concourse.mybir as mybir
    import concourse.tile as tile
    from concourse import bacc

    f32 = mybir.dt.float32
    f32r = mybir.dt.float32r
    bf16 = mybir.dt.bfloat16
    Exp = mybir.ActivationFunctionType.Exp

    nc = bacc.Bacc(
        "TRN2",
        target_bir_lowering=False,
        debug=False,
        enable_asserts=False,
        num_devices=NCORES,
    )

    xT_d = nc.dram_tensor("xT", [128, 4, 8, 512], bf16, kind="ExternalInput").ap()
    wq_d = nc.dram_tensor("wq", [128, 4, 8, 128], bf16, kind="ExternalInput").ap()
    wk_d = nc.dram_tensor("wk", [128, 4, 8, 128], bf16, kind="ExternalInput").ap()
    wv_d = nc.dram_tensor("wv", [128, 8, COLS], bf16, kind="ExternalInput").ap()
    bq_d = nc.dram_tensor("bq", [COLS], f32, kind="ExternalInput").ap()
    bk_d = nc.dram_tensor("bk", [COLS], f32, kind="ExternalInput").ap()
    bv_d = nc.dram_tensor("bv", [COLS], f32, kind="ExternalInput").ap()
    out_d = nc.dram_tensor("out", [S, COLS], f32, kind="ExternalOutput").ap()

    with tile.TileContext(nc) as tc:
        with (
            tc.tile_pool(name="consts", bufs=1) as consts,
            tc.tile_pool(name="wpool", bufs=1) as wpool,
            tc.tile_pool(name="qkp", bufs=1) as qkp,
            tc.tile_pool(name="vpool", bufs=1) as vpool,
            tc.tile_pool(name="xpool", bufs=1) as xpool,
            tc.tile_pool(name="epool", bufs=1) as epool,
            tc.tile_pool(name="opool", bufs=1) as opool,
            tc.tile_pool(name="psum", bufs=1, space="PSUM") as psum,
        ):
            from contextlib import contextmanager

            base = tc.cur_priority + 50
            att_cur = [base]
            fill_cur = [base + 8000]

            @contextmanager
            def band(cursor):
                off = tc.cur_priority - cursor[0]
                with tc.high_priority(offset=off):
                    yield
                    cursor[0] = tc.cur_priority

            # ---- constants ----
            with band(att_cur):
                warm = consts.tile([1, 1], f32)
                nc.vector.memset(warm, 0.0)
                nc.scalar.activation(warm, warm, Exp)  # pull ACT table load early
                # PE p-state warm-up: the tensor engine reaches full clock
                # only after ~3us of continuous execution. Run ~4us of
                # throwaway fp32 matmuls during the initial DMA window so the
                # first real projections start at full speed.
                wsrc = consts.tile([128, 512], f32, name="wsrc")
                nc.vector.memset(wsrc, 0.0)
                for _ in range(4):
                    wps = psum.tile([128, 4, 65], f32, tag="ctx", bufs=1, name="wps")
                    nc.tensor.matmul(
                        wps.rearrange("p t d -> p (t d)")[:, 0:260],
                        lhsT=wsrc[:, 0:128],
                        rhs=wsrc[:, 0:260],
                        start=True,
                        stop=True,
                    )

            with band(fill_cur):
                bq_t = consts.tile([128, 4], f32)
                bk_t = consts.tile([128, 4], f32)
                bv_s = consts.tile([1, COLS], f32)
                bvb = consts.tile([128, COLS], f32)
                nc.gpsimd.dma_start(out=bq_t, in_=bq_d.rearrange("(m p) -> p m", p=128))
                nc.gpsimd.dma_start(out=bk_t, in_=bk_d.rearrange("(m p) -> p m", p=128))
                nc.gpsimd.dma_start(out=bv_s, in_=bv_d[None, :])
                nc.gpsimd.partition_broadcast(bvb, bv_s)

                vt = [vpool.tile([128, 8, 65], bf16, name=f"vt{i}") for i in range(NKB)]
                for i in range(NKB):
                    nc.vector.memset(vt[i][:, :, 64:65], 1.0)

                wv_t = wpool.tile([128, 8, COLS], bf16, name="wv_t")

                # bf16 q/k transposed tiles per head pair m:
                # [128p = 2 heads x 64 dh, 2048 s]
                qt_b = [qkp.tile([128, S], bf16, name=f"qt{m}") for m in range(4)]
                kt_b = [qkp.tile([128, S], bf16, name=f"kt{m}") for m in range(4)]

            # ---- weight chunk ring (2 bufs per proj; quad1 reuses quad0's) ----
            wcur = {"q": {}, "k": {}}

            def load_w(proj, m, eng):
                w_d = wq_d if proj == "q" else wk_d
                wt = wpool.tile(
                    [128, 8, 128], bf16, tag=f"w{proj}", bufs=2, name=f"w{proj}{m}"
                )
                eng.dma_start(out=wt, in_=w_d[:, m, :, :])
                wcur[proj][m] = wt

            with band(fill_cur):
                # critical-path DMA order (all on the SP/HWDGE path; gpsimd
                # dma_start occupies the Pool engine ~1.1us per descriptor):
                # wk0, x0, wk1, x1, wq0, x2, wq1, x3, wv
                xt = []
                for c in range(4):
                    xc = xpool.tile([128, 8, 512], bf16, name=f"xt{c}")
                    xt.append(xc)

                def load_x(c):
                    nc.sync.dma_start(out=xt[c][:, 0:4, :], in_=xT_d[:, c, 0:4, :])
                    nc.sync.dma_start(out=xt[c][:, 4:8, :], in_=xT_d[:, c, 4:8, :])

                load_w("k", 0, nc.sync)
                load_w("q", 0, nc.sync)
                load_x(0)
                load_x(1)
                load_x(2)
                load_x(3)
                load_w("k", 1, nc.sync)
                load_w("q", 1, nc.sync)
                nc.sync.dma_start(out=wv_t, in_=wv_d)



            # ---- projection unit emitters (fill band) ----
            def proj_qk(proj, m, c):
                dst = qt_b[m] if proj == "q" else kt_b[m]
                bias_t = bq_t if proj == "q" else bk_t
                w = wcur[proj][m]
                ps = psum.tile([128, 512], f32, tag="qkv", bufs=2, name="psq")
                for j in range(8):
                    nc.tensor.matmul(
                        ps,
                        lhsT=w[:, j, :],
                        rhs=xt[c][:, j, :],
                        start=(j == 0),
                        stop=(j == 7),
                    )
                nc.vector.tensor_scalar_add(
                    dst[:, c * 512 : (c + 1) * 512], ps, bias_t[:, m : m + 1]
                )

            def proj_v(m, c, i):
                # V for head-pair m, s-chunk c, seq subchunk i -> vt[4c+i]
                ps = psum.tile([128, 512], f32, tag="qkv", bufs=2, name="psv")
                for j in range(8):
                    nc.tensor.matmul(
                        ps[:, 0:128],
                        lhsT=xt[c][:, j, i * 128 : (i + 1) * 128],
                        rhs=wv_t[:, j, m * 128 : (m + 1) * 128],
                        start=(j == 0),
                        stop=(j == 7),
                    )
                nc.vector.tensor_add(
                    vt[4 * c + i][:, 2 * m : 2 * m + 2, 0:64],
                    ps[:, 0:128].rearrange("p (h d) -> p h d", h=2),
                    bvb[:, m * 128 : (m + 1) * 128].rearrange("p (h d) -> p h d", h=2),
                )

            def proj_kq_fused(m, c):
                # k and q projections for pair m interleaved per x descriptor,
                # so both finish right after the last x chunk lands
                psk = psum.tile([128, 512], f32, tag="qkv", bufs=2, name="psk")
                psq = psum.tile([128, 512], f32, tag="qkv", bufs=2, name="psq2")
                for j in range(8):
                    nc.tensor.matmul(
                        psk, lhsT=wcur["k"][m][:, j, :], rhs=xt[c][:, j, :],
                        start=(j == 0), stop=(j == 7),
                    )
                    nc.tensor.matmul(
                        psq, lhsT=wcur["q"][m][:, j, :], rhs=xt[c][:, j, :],
                        start=(j == 0), stop=(j == 7),
                    )
                nc.vector.tensor_scalar_add(
                    kt_b[m][:, c * 512 : (c + 1) * 512], psk, bk_t[:, m : m + 1]
                )
                nc.vector.tensor_scalar_add(
                    qt_b[m][:, c * 512 : (c + 1) * 512], psq, bq_t[:, m : m + 1]
                )

            # projection emission order: priority mirrors consumption
            # (h-major groups: pair m's k/q before pair m's first head).
            proj_order = []
            for m in range(4):
                if m >= 2:
                    proj_order += [("wl", "k", m), ("wl", "q", m)]
                proj_order += [("kq", m, 0), ("k", m, 1), ("q", m, 1)]
                proj_order += [("k", m, 2), ("q", m, 2), ("k", m, 3), ("q", m, 3)]
                proj_order += [("v", m, c, i) for c in range(4) for i in range(4)]

            def emit_proj_all():
                with band(fill_cur):
                    for u in proj_order:
                        if u[0] == "wl":
                            load_w(u[1], u[2], nc.sync)
                        elif u[0] == "v":
                            proj_v(u[1], u[2], u[3])
                        elif u[0] == "kq":
                            proj_kq_fused(u[1], u[2])
                        else:
                            proj_qk(u[0], u[1], u[2])

            # ---- attention stream ----
            # group order: h-major. Each head's 4 q-blocks run consecutively;
            # h0/h1 share pair-0 k/q tiles, so the 21 units of h1 need no new
            # projections -- that window hides pair-1's projections, etc.
            groups = [(h, qb) for h in range(8) for qb in range(4)]
            slots = [(h, qb, kb) for (h, qb) in groups for kb in range(NKB)]
            units = []
            pos = 0
            widths = [3, 2]
            ui = 0
            while pos < len(slots):
                w = min(widths[ui % 2], len(slots) - pos)
                units.append(slots[pos : pos + w])
                pos += w
                ui += 1

            ctx_cur = [None]

            def emit_scores(u):
                unit = units[u]
                full = widths[u % 2]
                tag = "scA" if u % 2 == 0 else "scB"
                with band(att_cur):
                    sc = psum.tile([128, full, 512], f32, tag=tag, bufs=1, name="sc")
                    for i, (h, qb, kb) in enumerate(unit):
                        m, p0 = h // 2, 64 * (h % 2)
                        nc.tensor.matmul(
                            sc[:, i, :],
                            lhsT=kt_b[m][p0 : p0 + 64, kb * 128 : (kb + 1) * 128],
                            rhs=qt_b[m][p0 : p0 + 64, qb * 512 : (qb + 1) * 512],
                            start=True,
                            stop=True,
                        )
                return sc

            def emit_exp(u, sc):
                w = len(units[u])
                with band(att_cur):
                    ee = epool.tile([128, 3, 512], bf16, tag="e", bufs=26, name="ee")
                    nc.scalar.activation(
                        ee[:, 0:w, :], sc[:, 0:w, :], Exp, scale=EXP_SCALE
                    )
                return ee

            def emit_pv(u, ee):
                unit = units[u]
                with band(att_cur):
                    for i, (h, qb, kb) in enumerate(unit):
                        if kb == 0:
                            ctx_cur[0] = psum.tile(
                                [128, 4, 65], f32, tag="ctx", bufs=1, name="ctx"
                            )
                            nc.vector.memset(ctx_cur[0], 0.0)
                        ctx = ctx_cur[0]
                        for qt in range(4):
                            nc.tensor.matmul(
                                ctx[:, qt, :],
                                lhsT=ee[:, i, qt * 128 : (qt + 1) * 128],
                                rhs=vt[kb][:, h, :],
                                start=False,
                                stop=False,
                                skip_group_check=True,
                            )
                        if kb == NKB - 1:
                            rr = opool.tile([128, 4, 1], f32, tag="r", bufs=3, name="rr")
                            nc.vector.reciprocal(rr, ctx[:, :, 64:65])
                            ob = opool.tile([128, 4, 64], f32, tag="o", bufs=3, name="ob")
                            nc.vector.tensor_mul(
                                ob, ctx[:, :, 0:64], rr.broadcast_to([128, 4, 64])
                            )
                            nc.sync.dma_start(
                                out=out_d[
                                    qb * 512 : (qb + 1) * 512, h * 64 : (h + 1) * 64
                                ].rearrange("(t p) d -> p t d", p=128),
                                in_=ob,
                            )

            emit_proj_all()
            NU = len(units)
            scs = {0: emit_scores(0), 1: emit_scores(1)}
            for u in range(NU):
                ee = emit_exp(u, scs.pop(u))
                if u + 2 < NU:
                    scs[u + 2] = emit_scores(u + 2)
                emit_pv(u, ee)

    nc.compile()
    return nc


def _get_nc():
    if "nc" not in _CACHE:
        _CACHE["nc"] = _build()
    return _CACHE["nc"]


def _in_maps(x, Wq, bq, Wk, bk, Wv, bv):
    import ml_dtypes

    bf = ml_dtypes.bfloat16
    x = np.asarray(x, np.float32)
    maps = []
    for c in range(NCORES):
        b, hh = c // 2, c % 2
        cs = slice(hh * COLS, (hh + 1) * COLS)
        def warr(W):
            # [1024, 512] -> [128 p, 4 m, 8 j, 128 c]
            a = np.asarray(W, np.float32)[:, cs].astype(bf)
            return np.ascontiguousarray(
                a.reshape(8, 128, 4, 128).transpose(1, 2, 0, 3)
            )

        xTr = x[b].T.astype(bf).reshape(8, 128, 4, 512).transpose(1, 2, 0, 3)
        wvr = np.asarray(Wv, np.float32)[:, cs].astype(bf).reshape(8, 128, 512)
        maps.append(
            {
                "xT": np.ascontiguousarray(xTr),
                "wq": warr(Wq),
                "wk": warr(Wk),
                "wv": np.ascontiguousarray(wvr.transpose(1, 0, 2)),
                "bq": np.ascontiguousarray(np.asarray(bq, np.float32)[cs]),
                "bk": np.ascontiguousarray(np.asarray(bk, np.float32)[cs]),
                "bv": np.ascontiguousarray(np.asarray(bv, np.float32)[cs]),
            }
        )
    return maps


def _run(inputs, trace=False):
    from concourse import bass_utils

    nc = _get_nc()
    res = bass_utils.run_bass_kernel_spmd(
        nc,
        _in_maps(**inputs),
        core_ids=list(range(NCORES)),
        trace=trace,
    )
    out = np.empty((B, S, D), np.float32)
    for c in range(NCORES):
        b, hh = c // 2, c % 2
        out[b, :, hh * COLS : (hh + 1) * COLS] = res.results[c]["out"]
    return out, res


def kernel(**inputs):
    out, _ = _run(inputs, trace=False)
    return out


if __name__ == "__main__":
    _get_nc()
    print("build ok")


# revision 23
# speedup vs baseline: 1.1408x; 1.0032x over previous
"""Trainium2 Bass kernel for nn_AttentionLayer (B=4, S=2048, H=16, DH=64).

Sharding: 8 cores = 4 batches x 2 head-halves. Core c handles batch c//2,
heads (c%2)*8 .. (c%2)*8+8 (512 of the 1024 QKV columns).

Per-core structure (ACT-bound design, ~256us modeled):
  - Q/K projections (PE, fp32r per head-pair column chunk) write q/k
    transposed as bf16 via the DVE bias-add: qt/kt[m] [128p = 2 heads x
    64 dh, 2048 s].
  - V projection (PE, fp32r) -> vt[kb] [128 kpos, 8 heads, 65] bf16 with
    col 64 = 1.0 (PV accumulates the softmax denominator for free).
  - Attention stream: 512 slots; group = (head, 512-q block) with qb-minor
    group order (all heads' qb0 first) so q s-chunk demand is spread; slots
    = 16 k-blocks per group. Slots are packed into alternating 3-slot/2-slot
    PSUM score tiles; one exp (ACT) per tile (1536/1024 wide) is the
    metronome: ACT busy ~= 256us, PE ~= 247us.
  - Scores: bf16 matmul [64 dh contraction] -> scoresT [128 kpos, 512 q]
    per slot (bf16 q,k + bf16 E/V measured ~0.3% rel RMS; fp8 DoubleRow
    measured 2% = over the gate, hence bf16).
  - PV reoriented to ctx[q, d]: out [128 q, 65] per (slot, qtile) in bf16,
    accumulated over kb into a memset-zeroed PSUM bank per group
    (start=False + skip_group_check so 4 sub-bank groups share one bank).
  - Normalization: DVE reciprocal of ctx col 64 + per-qtile scalar multiply,
    DMA out via the Pool sequencer.
  PSUM: qkv 2 banks + scores 3+2 banks + ctx 1 bank = 8.
  Tile derives data dependencies from emission order, so all projections are
  emitted (low-priority fill band) before the attention stream; priorities
  interleave them into PE gaps at schedule time.
"""

import numpy as np

B, S, H, DH = 4, 2048, 16, 64
D = H * DH  # 1024
NCORES = 8
COLS = 512  # qkv columns per core (8 heads)
NKB = 16
EXP_SCALE = 0.125  # 1/sqrt(DH)

_CACHE = {}


def _build():
    import concourse.mybir as mybir
    import concourse.tile as tile
    from concourse import bacc

    f32 = mybir.dt.float32
    f32r = mybir.dt.float32r
    bf16 = mybir.dt.bfloat16
    Exp = mybir.ActivationFunctionType.Exp

    nc = bacc.Bacc(
        "TRN2",
        target_bir_lowering=False,
        debug=False,
        enable_asserts=False,
        num_devices=NCORES,
    )

    xT_d = nc.dram_tensor("xT", [128, 4, 8, 512], bf16, kind="ExternalInput").ap()
    wq_d = nc.dram_tensor("wq", [128, 4, 8, 128], bf16, kind="ExternalInput").ap()
    wk_d = nc.dram_tensor("wk", [128, 4, 8, 128], bf16, kind="ExternalInput").ap()
    wv_d = nc.dram_tensor("wv", [128, 8, COLS], bf16, kind="ExternalInput").ap()
    bq_d = nc.dram_tensor("bq", [COLS], f32, kind="ExternalInput").ap()
    bk_d = nc.dram_tensor("bk", [COLS], f32, kind="ExternalInput").ap()
    bv_d = nc.dram_tensor("bv", [COLS], f32, kind="ExternalInput").ap()
    out_d = nc.dram_tensor("out", [S, COLS], f32, kind="ExternalOutput").ap()

    with tile.TileContext(nc) as tc:
        with (
            tc.tile_pool(name="consts", bufs=1) as consts,
            tc.tile_pool(name="wpool", bufs=1) as wpool,
            tc.tile_pool(name="qkp", bufs=1) as qkp,
            tc.tile_pool(name="vpool", bufs=1) as vpool,
            tc.tile_pool(name="xpool", bufs=1) as xpool,
            tc.tile_pool(name="epool", bufs=1) as epool,
            tc.tile_pool(name="opool", bufs=1) as opool,
            tc.tile_pool(name="psum", bufs=1, space="PSUM") as psum,
        ):
            from contextlib import contextmanager

            base = tc.cur_priority + 50
            att_cur = [base]
            fill_cur = [base + 8000]

            @contextmanager
            def band(cursor):
                off = tc.cur_priority - cursor[0]
                with tc.high_priority(offset=off):
                    yield
                    cursor[0] = tc.cur_priority

            # ---- constants ----
            with band(att_cur):
                warm = consts.tile([1, 1], f32)
                nc.vector.memset(warm, 0.0)
                nc.scalar.activation(warm, warm, Exp)  # pull ACT table load early
                # PE p-state warm-up: the tensor engine reaches full clock
                # only after ~3us of continuous execution. Run ~4us of
                # throwaway fp32 matmuls during the initial DMA window so the
                # first real projections start at full speed.
                wsrc = consts.tile([128, 128], f32, name="wsrc")
                nc.vector.memset(wsrc, 0.0)
                for _ in range(10):
                    wps = psum.tile([128, 4, 65], f32, tag="ctx", bufs=1, name="wps")
                    nc.tensor.matmul(
                        wps.rearrange("p t d -> p (t d)")[:, 0:64],
                        lhsT=wsrc,
                        rhs=wsrc[:, 0:64],
                        start=True,
                        stop=True,
                    )

            with band(fill_cur):
                bq_t = consts.tile([128, 4], f32)
                bk_t = consts.tile([128, 4], f32)
                bv_s = consts.tile([1, COLS], f32)
                bvb = consts.tile([128, COLS], f32)
                nc.gpsimd.dma_start(out=bq_t, in_=bq_d.rearrange("(m p) -> p m", p=128))
                nc.gpsimd.dma_start(out=bk_t, in_=bk_d.rearrange("(m p) -> p m", p=128))
                nc.gpsimd.dma_start(out=bv_s, in_=bv_d[None, :])
                nc.gpsimd.partition_broadcast(bvb, bv_s)

                vt = [vpool.tile([128, 8, 65], bf16, name=f"vt{i}") for i in range(NKB)]
                for i in range(NKB):
                    nc.vector.memset(vt[i][:, :, 64:65], 1.0)

                wv_t = wpool.tile([128, 8, COLS], bf16, name="wv_t")

                # bf16 q/k transposed tiles per head pair m:
                # [128p = 2 heads x 64 dh, 2048 s]
                qt_b = [qkp.tile([128, S], bf16, name=f"qt{m}") for m in range(4)]
                kt_b = [qkp.tile([128, S], bf16, name=f"kt{m}") for m in range(4)]

            # ---- weight chunk ring (2 bufs per proj; quad1 reuses quad0's) ----
            wcur = {"q": {}, "k": {}}

            def load_w(proj, m, eng):
                w_d = wq_d if proj == "q" else wk_d
                wt = wpool.tile(
                    [128, 8, 128], bf16, tag=f"w{proj}", bufs=2, name=f"w{proj}{m}"
                )
                eng.dma_start(out=wt, in_=w_d[:, m, :, :])
                wcur[proj][m] = wt

            with band(fill_cur):
                # critical-path DMA order (all on the SP/HWDGE path; gpsimd
                # dma_start occupies the Pool engine ~1.1us per descriptor):
                # wk0, x0, wk1, x1, wq0, x2, wq1, x3, wv
                xt = []
                for c in range(4):
                    xc = xpool.tile([128, 8, 512], bf16, name=f"xt{c}")
                    xt.append(xc)

                def load_x(c):
                    nc.sync.dma_start(out=xt[c][:, 0:4, :], in_=xT_d[:, c, 0:4, :])
                    nc.sync.dma_start(out=xt[c][:, 4:8, :], in_=xT_d[:, c, 4:8, :])

                load_w("k", 0, nc.sync)
                load_w("q", 0, nc.sync)
                load_x(0)
                load_x(1)
                load_x(2)
                load_x(3)
                load_w("k", 1, nc.sync)
                load_w("q", 1, nc.sync)
                nc.sync.dma_start(out=wv_t, in_=wv_d)



            def proj_kq_fused(m, c):
                # k and q projections for pair m interleaved per x descriptor,
                # so both finish right after the last x chunk lands
                psk = psum.tile([128, 512], f32, tag="qkv", bufs=2, name="psk")
                psq = psum.tile([128, 512], f32, tag="qkv", bufs=2, name="psq2")
                for j in range(8):
                    nc.tensor.matmul(
                        psk, lhsT=wcur["k"][m][:, j, :], rhs=xt[c][:, j, :],
                        start=(j == 0), stop=(j == 7),
                    )
                    nc.tensor.matmul(
                        psq, lhsT=wcur["q"][m][:, j, :], rhs=xt[c][:, j, :],
                        start=(j == 0), stop=(j == 7),
                    )
                nc.vector.tensor_scalar_add(
                    kt_b[m][:, c * 512 : (c + 1) * 512], psk, bk_t[:, m : m + 1]
                )
                nc.vector.tensor_scalar_add(
                    qt_b[m][:, c * 512 : (c + 1) * 512], psq, bq_t[:, m : m + 1]
                )

            # ---- projection unit emitters (fill band) ----
            def proj_qk(proj, m, c):
                dst = qt_b[m] if proj == "q" else kt_b[m]
                bias_t = bq_t if proj == "q" else bk_t
                w = wcur[proj][m]
                ps = psum.tile([128, 512], f32, tag="qkv", bufs=2, name="psq")
                for j in range(8):
                    nc.tensor.matmul(
                        ps,
                        lhsT=w[:, j, :],
                        rhs=xt[c][:, j, :],
                        start=(j == 0),
                        stop=(j == 7),
                    )
                nc.vector.tensor_scalar_add(
                    dst[:, c * 512 : (c + 1) * 512], ps, bias_t[:, m : m + 1]
                )

            def proj_v(m, c, i):
                # V for head-pair m, s-chunk c, seq subchunk i -> vt[4c+i]
                ps = psum.tile([128, 512], f32, tag="qkv", bufs=2, name="psv")
                for j in range(8):
                    nc.tensor.matmul(
                        ps[:, 0:128],
                        lhsT=xt[c][:, j, i * 128 : (i + 1) * 128],
                        rhs=wv_t[:, j, m * 128 : (m + 1) * 128],
                        start=(j == 0),
                        stop=(j == 7),
                    )
                nc.vector.tensor_add(
                    vt[4 * c + i][:, 2 * m : 2 * m + 2, 0:64],
                    ps[:, 0:128].rearrange("p (h d) -> p h d", h=2),
                    bvb[:, m * 128 : (m + 1) * 128].rearrange("p (h d) -> p h d", h=2),
                )

            # projection emission order: priority mirrors consumption
            # (h-major groups: pair m's k/q before pair m's first head).
            proj_order = []
            for m in range(4):
                if m >= 2:
                    proj_order += [("wl", "k", m), ("wl", "q", m)]
                proj_order += [("kq", m, 0), ("k", m, 1), ("q", m, 1)]
                proj_order += [("k", m, 2), ("q", m, 2), ("k", m, 3), ("q", m, 3)]
                proj_order += [("v", m, c, i) for c in range(4) for i in range(4)]

            def emit_proj_all():
                with band(fill_cur):
                    for u in proj_order:
                        if u[0] == "wl":
                            load_w(u[1], u[2], nc.sync)
                        elif u[0] == "v":
                            proj_v(u[1], u[2], u[3])
                        elif u[0] == "kq":
                            proj_kq_fused(u[1], u[2])
                        else:
                            proj_qk(u[0], u[1], u[2])

            # ---- attention stream ----
            # group order: h-major. Each head's 4 q-blocks run consecutively;
            # h0/h1 share pair-0 k/q tiles, so the 21 units of h1 need no new
            # projections -- that window hides pair-1's projections, etc.
            groups = [(h, qb) for h in range(8) for qb in range(4)]
            slots = [(h, qb, kb) for (h, qb) in groups for kb in range(NKB)]
            units = []
            pos = 0
            ui = 0
            prefix = [1, 2]
            while pos < len(slots):
                if ui < len(prefix):
                    w = prefix[ui]
                else:
                    w = 3 if ui % 2 == 0 else 2
                w = min(w, len(slots) - pos)
                units.append(slots[pos : pos + w])
                pos += w
                ui += 1

            ctx_cur = [None]

            def emit_scores(u):
                unit = units[u]
                full = 3 if u % 2 == 0 else 2
                tag = "scA" if u % 2 == 0 else "scB"
                with band(att_cur):
                    sc = psum.tile([128, full, 512], f32, tag=tag, bufs=1, name="sc")
                    for i, (h, qb, kb) in enumerate(unit):
                        m, p0 = h // 2, 64 * (h % 2)
                        nc.tensor.matmul(
                            sc[:, i, :],
                            lhsT=kt_b[m][p0 : p0 + 64, kb * 128 : (kb + 1) * 128],
                            rhs=qt_b[m][p0 : p0 + 64, qb * 512 : (qb + 1) * 512],
                            start=True,
                            stop=True,
                        )
                return sc

            def emit_exp(u, sc):
                w = len(units[u])
                with band(att_cur):
                    ee = epool.tile([128, 3, 512], bf16, tag="e", bufs=26, name="ee")
                    nc.scalar.activation(
                        ee[:, 0:w, :], sc[:, 0:w, :], Exp, scale=EXP_SCALE
                    )
                return ee

            def emit_pv(u, ee):
                unit = units[u]
                with band(att_cur):
                    for i, (h, qb, kb) in enumerate(unit):
                        if kb == 0:
                            ctx_cur[0] = psum.tile(
                                [128, 4, 65], f32, tag="ctx", bufs=1, name="ctx"
                            )
                            nc.vector.memset(ctx_cur[0], 0.0)
                        ctx = ctx_cur[0]
                        for qt in range(4):
                            nc.tensor.matmul(
                                ctx[:, qt, :],
                                lhsT=ee[:, i, qt * 128 : (qt + 1) * 128],
                                rhs=vt[kb][:, h, :],
                                start=False,
                                stop=False,
                                skip_group_check=True,
                            )
                        if kb == NKB - 1:
                            rr = opool.tile([128, 4, 1], f32, tag="r", bufs=3, name="rr")
                            nc.vector.reciprocal(rr, ctx[:, :, 64:65])
                            ob = opool.tile([128, 4, 64], f32, tag="o", bufs=3, name="ob")
                            nc.vector.tensor_mul(
                                ob, ctx[:, :, 0:64], rr.broadcast_to([128, 4, 64])
                            )
                            nc.sync.dma_start(
                                out=out_d[
                                    qb * 512 : (qb + 1) * 512, h * 64 : (h + 1) * 64
                                ].rearrange("(t p) d -> p t d", p=128),
                                in_=ob,
                            )

            emit_proj_all()
            NU = len(units)
            scs = {0: emit_scores(0), 1: emit_scores(1)}
            for u in range(NU):
                ee = emit_exp(u, scs.pop(u))
                if u + 2 < NU:
                    scs[u + 2] = emit_scores(u + 2)
                emit_pv(u, ee)

    nc.compile()
    return nc


def _get_nc():
    if "nc" not in _CACHE:
        _CACHE["nc"] = _build()
    return _CACHE["nc"]


def _in_maps(x, Wq, bq, Wk, bk, Wv, bv):
    import ml_dtypes

    bf = ml_dtypes.bfloat16
    x = np.asarray(x, np.float32)
    maps = []
    for c in range(NCORES):
        b, hh = c // 2, c % 2
        cs = slice(hh * COLS, (hh + 1) * COLS)
        def warr(W):
            # [1024, 512] -> [128 p, 4 m, 8 j, 128 c]
            a = np.asarray(W, np.float32)[:, cs].astype(bf)
            return np.ascontiguousarray(
                a.reshape(8, 128, 4, 128).transpose(1, 2, 0, 3)
            )

        xTr = x[b].T.astype(bf).reshape(8, 128, 4, 512).transpose(1, 2, 0, 3)
        wvr = np.asarray(Wv, np.float32)[:, cs].astype(bf).reshape(8, 128, 512)
        maps.append(
            {
                "xT": np.ascontiguousarray(xTr),
                "wq": warr(Wq),
                "wk": warr(Wk),
                "wv": np.ascontiguousarray(wvr.transpose(1, 0, 2)),
                "bq": np.ascontiguousarray(np.asarray(bq, np.float32)[cs]),
                "bk": np.ascontiguousarray(np.asarray(bk, np.float32)[cs]),
                "bv": np.ascontiguousarray(np.asarray(bv, np.float32)[cs]),
            }
        )
    return maps


def _run(inputs, trace=False):
    from concourse import bass_utils

    nc = _get_nc()
    res = bass_utils.run_bass_kernel_spmd(
        nc,
        _in_maps(**inputs),
        core_ids=list(range(NCORES)),
        trace=trace,
    )
    out = np.empty((B, S, D), np.float32)
    for c in range(NCORES):
        b, hh = c // 2, c % 2
        out[b, :, hh * COLS : (hh + 1) * COLS] = res.results[c]["out"]
    return out, res


def kernel(**inputs):
    out, _ = _run(inputs, trace=False)
    return out


if __name__ == "__main__":
    _get_nc()
    print("build ok")


# revision 34
# speedup vs baseline: 1.1461x; 1.0047x over previous
"""Trainium2 Bass kernel for nn_AttentionLayer (B=4, S=2048, H=16, DH=64).

Sharding: 8 cores = 4 batches x 2 head-halves. Core c handles batch c//2,
heads (c%2)*8 .. (c%2)*8+8 (512 of the 1024 QKV columns).

Per-core structure (ACT-bound design, ~256us modeled):
  - Q/K projections (PE, fp32r per head-pair column chunk) write q/k
    transposed as bf16 via the DVE bias-add: qt/kt[m] [128p = 2 heads x
    64 dh, 2048 s].
  - V projection (PE, fp32r) -> vt[kb] [128 kpos, 8 heads, 65] bf16 with
    col 64 = 1.0 (PV accumulates the softmax denominator for free).
  - Attention stream: 512 slots; group = (head, 512-q block) with qb-minor
    group order (all heads' qb0 first) so q s-chunk demand is spread; slots
    = 16 k-blocks per group. Slots are packed into alternating 3-slot/2-slot
    PSUM score tiles; one exp (ACT) per tile (1536/1024 wide) is the
    metronome: ACT busy ~= 256us, PE ~= 247us.
  - Scores: bf16 matmul [64 dh contraction] -> scoresT [128 kpos, 512 q]
    per slot (bf16 q,k + bf16 E/V measured ~0.3% rel RMS; fp8 DoubleRow
    measured 2% = over the gate, hence bf16).
  - PV reoriented to ctx[q, d]: out [128 q, 65] per (slot, qtile) in bf16,
    accumulated over kb into a memset-zeroed PSUM bank per group
    (start=False + skip_group_check so 4 sub-bank groups share one bank).
  - Normalization: DVE reciprocal of ctx col 64 + per-qtile scalar multiply,
    DMA out via the Pool sequencer.
  PSUM: qkv 2 banks + scores 3+2 banks + ctx 1 bank = 8.
  Tile derives data dependencies from emission order, so all projections are
  emitted (low-priority fill band) before the attention stream; priorities
  interleave them into PE gaps at schedule time.
"""

import numpy as np

B, S, H, DH = 4, 2048, 16, 64
D = H * DH  # 1024
NCORES = 8
COLS = 512  # qkv columns per core (8 heads)
NKB = 16
EXP_SCALE = 0.125  # 1/sqrt(DH)

_CACHE = {}


def _build():
    import concourse.mybir as mybir
    import concourse.tile as tile
    from concourse import bacc

    f32 = mybir.dt.float32
    f32r = mybir.dt.float32r
    bf16 = mybir.dt.bfloat16
    Exp = mybir.ActivationFunctionType.Exp

    nc = bacc.Bacc(
        "TRN2",
        target_bir_lowering=False,
        debug=False,
        enable_asserts=False,
        num_devices=NCORES,
    )

    xT_d = nc.dram_tensor("xT", [128, 4, 8, 512], bf16, kind="ExternalInput").ap()
    wq_d = nc.dram_tensor("wq", [128, 4, 8, 128], bf16, kind="ExternalInput").ap()
    wk_d = nc.dram_tensor("wk", [128, 4, 8, 128], bf16, kind="ExternalInput").ap()
    wv_d = nc.dram_tensor("wv", [128, 8, COLS], bf16, kind="ExternalInput").ap()
    bq_d = nc.dram_tensor("bq", [COLS], f32, kind="ExternalInput").ap()
    bk_d = nc.dram_tensor("bk", [COLS], f32, kind="ExternalInput").ap()
    bv_d = nc.dram_tensor("bv", [COLS], f32, kind="ExternalInput").ap()
    out_d = nc.dram_tensor("out", [S, COLS], f32, kind="ExternalOutput").ap()

    with tile.TileContext(nc) as tc:
        with (
            tc.tile_pool(name="consts", bufs=1) as consts,
            tc.tile_pool(name="wpool", bufs=1) as wpool,
            tc.tile_pool(name="qkp", bufs=1) as qkp,
            tc.tile_pool(name="vpool", bufs=1) as vpool,
            tc.tile_pool(name="xpool", bufs=1) as xpool,
            tc.tile_pool(name="epool", bufs=1) as epool,
            tc.tile_pool(name="opool", bufs=1) as opool,
            tc.tile_pool(name="psum", bufs=1, space="PSUM") as psum,
        ):
            from contextlib import contextmanager

            base = tc.cur_priority + 50
            att_cur = [base]
            fill_cur = [base + 8000]

            @contextmanager
            def band(cursor):
                off = tc.cur_priority - cursor[0]
                with tc.high_priority(offset=off):
                    yield
                    cursor[0] = tc.cur_priority

            # ---- constants ----
            with band(att_cur):
                warm = consts.tile([1, 1], f32)
                nc.vector.memset(warm, 0.0)
                nc.scalar.activation(warm, warm, Exp)  # pull ACT table load early
                # PE p-state warm-up: the tensor engine reaches full clock
                # only after ~3us of continuous execution. Run ~4us of
                # throwaway fp32 matmuls during the initial DMA window so the
                # first real projections start at full speed.
                wsrc = consts.tile([128, 128], f32, name="wsrc")
                nc.vector.memset(wsrc, 0.0)
                for _ in range(10):
                    wps = psum.tile([128, 4, 65], f32, tag="ctx", bufs=1, name="wps")
                    nc.tensor.matmul(
                        wps.rearrange("p t d -> p (t d)")[:, 0:64],
                        lhsT=wsrc,
                        rhs=wsrc[:, 0:64],
                        start=True,
                        stop=True,
                    )

            with band(fill_cur):
                bq_t = consts.tile([128, 4], f32)
                bk_t = consts.tile([128, 4], f32)
                bv_s = consts.tile([1, COLS], f32)
                bvb = consts.tile([128, COLS], f32)
                nc.gpsimd.dma_start(out=bq_t, in_=bq_d.rearrange("(m p) -> p m", p=128))
                nc.gpsimd.dma_start(out=bk_t, in_=bk_d.rearrange("(m p) -> p m", p=128))
                nc.gpsimd.dma_start(out=bv_s, in_=bv_d[None, :])
                nc.gpsimd.partition_broadcast(bvb, bv_s)

                vt = [vpool.tile([128, 8, 65], bf16, name=f"vt{i}") for i in range(NKB)]
                for i in range(NKB):
                    nc.vector.memset(vt[i][:, :, 64:65], 1.0)

                wv_t = wpool.tile([128, 8, COLS], bf16, name="wv_t")

                # bf16 q/k transposed tiles per head pair m:
                # [128p = 2 heads x 64 dh, 2048 s]
                qt_b = [qkp.tile([128, S], bf16, name=f"qt{m}") for m in range(4)]
                kt_b = [qkp.tile([128, S], bf16, name=f"kt{m}") for m in range(4)]

            # ---- weight chunk ring (2 bufs per proj; quad1 reuses quad0's) ----
            wcur = {"q": {}, "k": {}}

            def load_w(proj, m, eng):
                w_d = wq_d if proj == "q" else wk_d
                wt = wpool.tile(
                    [128, 8, 128], bf16, tag=f"w{proj}", bufs=2, name=f"w{proj}{m}"
                )
                eng.dma_start(out=wt, in_=w_d[:, m, :, :])
                wcur[proj][m] = wt

            with band(fill_cur):
                # critical-path DMA order (all on the SP/HWDGE path; gpsimd
                # dma_start occupies the Pool engine ~1.1us per descriptor):
                # wk0, x0, wk1, x1, wq0, x2, wq1, x3, wv
                xt = []
                for c in range(4):
                    xc = xpool.tile([128, 8, 512], bf16, name=f"xt{c}")
                    xt.append(xc)

                def load_x(c):
                    for j0 in range(0, 8, 2):
                        nc.sync.dma_start(
                            out=xt[c][:, j0 : j0 + 2, :], in_=xT_d[:, c, j0 : j0 + 2, :]
                        )

                load_w("k", 0, nc.sync)
                load_w("q", 0, nc.sync)
                load_x(0)
                load_x(1)
                load_x(2)
                load_x(3)
                load_w("k", 1, nc.sync)
                load_w("q", 1, nc.sync)
                nc.sync.dma_start(out=wv_t, in_=wv_d)



            def proj_kq_fused(m, c):
                # k and q projections for pair m interleaved per x descriptor,
                # so both finish right after the last x chunk lands
                psk = psum.tile([128, 512], f32, tag="qkv", bufs=2, name="psk")
                psq = psum.tile([128, 512], f32, tag="qkv", bufs=2, name="psq2")
                for j in range(8):
                    nc.tensor.matmul(
                        psk, lhsT=wcur["k"][m][:, j, :], rhs=xt[c][:, j, :],
                        start=(j == 0), stop=(j == 7),
                    )
                    nc.tensor.matmul(
                        psq, lhsT=wcur["q"][m][:, j, :], rhs=xt[c][:, j, :],
                        start=(j == 0), stop=(j == 7),
                    )
                nc.vector.tensor_scalar_add(
                    kt_b[m][:, c * 512 : (c + 1) * 512], psk, bk_t[:, m : m + 1]
                )
                nc.vector.tensor_scalar_add(
                    qt_b[m][:, c * 512 : (c + 1) * 512], psq, bq_t[:, m : m + 1]
                )

            # ---- projection unit emitters (fill band) ----
            def proj_qk(proj, m, c):
                dst = qt_b[m] if proj == "q" else kt_b[m]
                bias_t = bq_t if proj == "q" else bk_t
                w = wcur[proj][m]
                ps = psum.tile([128, 512], f32, tag="qkv", bufs=2, name="psq")
                for j in range(8):
                    nc.tensor.matmul(
                        ps,
                        lhsT=w[:, j, :],
                        rhs=xt[c][:, j, :],
                        start=(j == 0),
                        stop=(j == 7),
                    )
                nc.vector.tensor_scalar_add(
                    dst[:, c * 512 : (c + 1) * 512], ps, bias_t[:, m : m + 1]
                )

            def proj_v(m, c, i):
                # V for head-pair m, s-chunk c, seq subchunk i -> vt[4c+i]
                ps = psum.tile([128, 512], f32, tag="qkv", bufs=2, name="psv")
                for j in range(8):
                    nc.tensor.matmul(
                        ps[:, 0:128],
                        lhsT=xt[c][:, j, i * 128 : (i + 1) * 128],
                        rhs=wv_t[:, j, m * 128 : (m + 1) * 128],
                        start=(j == 0),
                        stop=(j == 7),
                    )
                nc.vector.tensor_add(
                    vt[4 * c + i][:, 2 * m : 2 * m + 2, 0:64],
                    ps[:, 0:128].rearrange("p (h d) -> p h d", h=2),
                    bvb[:, m * 128 : (m + 1) * 128].rearrange("p (h d) -> p h d", h=2),
                )

            # projection emission order: priority mirrors consumption
            # (h-major groups: pair m's k/q before pair m's first head).
            proj_order = []
            for m in range(4):
                if m >= 2:
                    proj_order += [("wl", "k", m), ("wl", "q", m)]
                proj_order += [("kq", m, 0), ("k", m, 1), ("q", m, 1)]
                proj_order += [("k", m, 2), ("q", m, 2), ("k", m, 3), ("q", m, 3)]
                proj_order += [("v", m, c, i) for c in range(4) for i in range(4)]

            def emit_proj_all():
                with band(fill_cur):
                    for u in proj_order:
                        if u[0] == "wl":
                            load_w(u[1], u[2], nc.sync)
                        elif u[0] == "v":
                            proj_v(u[1], u[2], u[3])
                        elif u[0] == "kq":
                            proj_kq_fused(u[1], u[2])
                        else:
                            proj_qk(u[0], u[1], u[2])

            # ---- attention stream ----
            # group order: h-major. Each head's 4 q-blocks run consecutively;
            # h0/h1 share pair-0 k/q tiles, so the 21 units of h1 need no new
            # projections -- that window hides pair-1's projections, etc.
            groups = [(h, qb) for h in range(8) for qb in range(4)]
            slots = [(h, qb, kb) for (h, qb) in groups for kb in range(NKB)]
            units = []
            pos = 0
            ui = 0
            prefix = [2, 2]
            while pos < len(slots):
                if ui < len(prefix):
                    w = prefix[ui]
                else:
                    w = 3 if ui % 2 == 0 else 2
                w = min(w, len(slots) - pos)
                units.append(slots[pos : pos + w])
                pos += w
                ui += 1

            ctx_cur = [None]

            def emit_scores(u):
                unit = units[u]
                full = 3 if u % 2 == 0 else 2
                tag = "scA" if u % 2 == 0 else "scB"
                with band(att_cur):
                    sc = psum.tile([128, full, 512], f32, tag=tag, bufs=1, name="sc")
                    for i, (h, qb, kb) in enumerate(unit):
                        m, p0 = h // 2, 64 * (h % 2)
                        nc.tensor.matmul(
                            sc[:, i, :],
                            lhsT=kt_b[m][p0 : p0 + 64, kb * 128 : (kb + 1) * 128],
                            rhs=qt_b[m][p0 : p0 + 64, qb * 512 : (qb + 1) * 512],
                            start=True,
                            stop=True,
                        )
                return sc

            def emit_exp(u, sc):
                w = len(units[u])
                with band(att_cur):
                    ee = epool.tile([128, 3, 512], bf16, tag="e", bufs=28, name="ee")
                    nc.scalar.activation(
                        ee[:, 0:w, :], sc[:, 0:w, :], Exp, scale=EXP_SCALE
                    )
                return ee

            def emit_pv(u, ee):
                unit = units[u]
                with band(att_cur):
                    for i, (h, qb, kb) in enumerate(unit):
                        if kb == 0:
                            ctx_cur[0] = psum.tile(
                                [128, 4, 65], f32, tag="ctx", bufs=1, name="ctx"
                            )
                            nc.vector.memset(ctx_cur[0], 0.0)
                        ctx = ctx_cur[0]
                        for qt in range(4):
                            nc.tensor.matmul(
                                ctx[:, qt, :],
                                lhsT=ee[:, i, qt * 128 : (qt + 1) * 128],
                                rhs=vt[kb][:, h, :],
                                start=False,
                                stop=False,
                                skip_group_check=True,
                            )
                        if kb == NKB - 1:
                            rr = opool.tile([128, 4, 1], f32, tag="r", bufs=3, name="rr")
                            nc.vector.reciprocal(rr, ctx[:, :, 64:65])
                            ob = opool.tile([128, 4, 64], f32, tag="o", bufs=3, name="ob")
                            nc.vector.tensor_mul(
                                ob, ctx[:, :, 0:64], rr.broadcast_to([128, 4, 64])
                            )
                            nc.sync.dma_start(
                                out=out_d[
                                    qb * 512 : (qb + 1) * 512, h * 64 : (h + 1) * 64
                                ].rearrange("(t p) d -> p t d", p=128),
                                in_=ob,
                            )

            emit_proj_all()
            NU = len(units)
            scs = {0: emit_scores(0), 1: emit_scores(1)}
            for u in range(NU):
                ee = emit_exp(u, scs.pop(u))
                if u + 2 < NU:
                    scs[u + 2] = emit_scores(u + 2)
                emit_pv(u, ee)

    nc.compile()
    return nc


def _get_nc():
    if "nc" not in _CACHE:
        _CACHE["nc"] = _build()
    return _CACHE["nc"]


def _in_maps(x, Wq, bq, Wk, bk, Wv, bv):
    import ml_dtypes

    bf = ml_dtypes.bfloat16
    x = np.asarray(x, np.float32)
    maps = []
    for c in range(NCORES):
        b, hh = c // 2, c % 2
        cs = slice(hh * COLS, (hh + 1) * COLS)
        def warr(W):
            # [1024, 512] -> [128 p, 4 m, 8 j, 128 c]
            a = np.asarray(W, np.float32)[:, cs].astype(bf)
            return np.ascontiguousarray(
                a.reshape(8, 128, 4, 128).transpose(1, 2, 0, 3)
            )

        xTr = x[b].T.astype(bf).reshape(8, 128, 4, 512).transpose(1, 2, 0, 3)
        wvr = np.asarray(Wv, np.float32)[:, cs].astype(bf).reshape(8, 128, 512)
        maps.append(
            {
                "xT": np.ascontiguousarray(xTr),
                "wq": warr(Wq),
                "wk": warr(Wk),
                "wv": np.ascontiguousarray(wvr.transpose(1, 0, 2)),
                "bq": np.ascontiguousarray(np.asarray(bq, np.float32)[cs]),
                "bk": np.ascontiguousarray(np.asarray(bk, np.float32)[cs]),
                "bv": np.ascontiguousarray(np.asarray(bv, np.float32)[cs]),
            }
        )
    return maps


def _run(inputs, trace=False):
    from concourse import bass_utils

    nc = _get_nc()
    res = bass_utils.run_bass_kernel_spmd(
        nc,
        _in_maps(**inputs),
        core_ids=list(range(NCORES)),
        trace=trace,
    )
    out = np.empty((B, S, D), np.float32)
    for c in range(NCORES):
        b, hh = c // 2, c % 2
        out[b, :, hh * COLS : (hh + 1) * COLS] = res.results[c]["out"]
    return out, res


def kernel(**inputs):
    out, _ = _run(inputs, trace=False)
    return out


if __name__ == "__main__":
    _get_nc()
    print("build ok")
